# revision 1
# baseline (speedup 1.0000x reference)
"""Bahdanau additive attention on 8 TRN2 NeuronCores (data-parallel over batch).

Per batch item b:
    pre[a, t]  = sum_d Wh[a, d] h[t, d]  + bias_b[a],
                 bias_b = Wz @ z_b + bias          (z broadcast over t folds
                                                    into a per-partition bias)
    k          = tanh(pre)                         (ACT, bias fused)
    score[t]   = sum_a v[a] k[a, t]                (PE matvec, contraction on a)
    alpha      = softmax(score)                    (no max-subtraction needed:
                                                    |score| <= sum|v| ~ 40)
    C[d]       = sum_t alpha[t] h[t, d]            (DVE: hT ⊙ alpha-bcast, reduce)

Layout: pre is [a(partitions) x t(free)] so the z/b bias is a per-partition
scalar fused into the tanh activation, score is a PE matvec, softmax runs
on a [1, T] row, and C reduces along the free axis of hT on the vector
engine (alpha replicated across partitions via a stride-0 DMA from a DRAM
scratch row).

h enters the tensor engine with d on partitions (hT). f32 has no DMA
transpose, so each batch is cast to bf16, bounced to DRAM scratch
quadrants, and re-read through the DMA XBAR transpose (DRAM->SBUF is the
production-validated path; SBUF->SBUF transpose deadlocks).

Sharding: batch 64 -> 8 cores x 8 batches. W/b/v replicated. No
collectives; the host concatenates the per-core [8, 1024] outputs.
"""

import os
import sys

for _p in ("/opt/trn_rl_repo", "/root/.axon_site/_ro/trn_rl_repo"):
    if os.path.isdir(_p) and _p not in sys.path:
        sys.path.insert(0, _p)

import numpy as np

P = 128
NCORES = 8
B, T, D = 64, 2048, 1024
BL = B // NCORES          # local batches per core
TB = T // P               # 16 t-blocks
DO = D // P               # 8 128-blocks in D (for d, a, dz alike)
AH = 2
NH = D // AH

_graph_cache = None


def _build(BL=BL, T=T, D=D, finalize=True):
    DO = D // P
    TQ = max(1, T // 512)     # DRAM-scratch quadrants
    QR = T // TQ              # rows per quadrant
    QB = QR // P              # t-blocks per quadrant
    CH = min(512, T)          # matmul t-chunk (= one PSUM bank)
    NCH = T // CH

    import concourse.bass as bass
    import concourse.tile as tile
    import concourse.mybir as mybir
    from concourse import bacc

    F32 = mybir.dt.float32
    BF16 = mybir.dt.bfloat16
    Tanh = mybir.ActivationFunctionType.Tanh
    Exp = mybir.ActivationFunctionType.Exp
    Add = mybir.AluOpType.add

    nc = bacc.Bacc(None, target_bir_lowering=False)
    Z_ext = nc.declare_dram_parameter("Z_st", [BL, D], F32, isOutput=False)
    h_ext = nc.declare_dram_parameter("h_n_state", [BL, T, D], F32, isOutput=False)
    W_ext = nc.declare_dram_parameter("W", [D, 2 * D], F32, isOutput=False)
    b_ext = nc.declare_dram_parameter("b", [D], F32, isOutput=False)
    v_ext = nc.declare_dram_parameter("v", [D], F32, isOutput=False)
    out_ext = nc.declare_dram_parameter("out", [BL, D], F32, isOutput=True)

    with tile.TileContext(nc) as tc:
        with (
            tc.tile_pool(name="stage", bufs=2) as stage_pool,
            tc.tile_pool(name="cbf", bufs=2) as cbf_pool,
            tc.tile_pool(name="hT", bufs=2) as hT_pool,
            tc.tile_pool(name="kc", bufs=2) as kc_pool,
            tc.tile_pool(name="tmpc", bufs=1) as tmpc_pool,
            tc.tile_pool(name="const", bufs=1) as const_pool,
            tc.tile_pool(name="misc", bufs=2) as misc_pool,
            tc.tile_pool(name="dram", bufs=2, space="DRAM") as dram_pool,
            tc.tile_pool(name="ppre", bufs=3, space="PSUM") as ppre_pool,
            tc.tile_pool(name="pscore", bufs=2, space="PSUM") as pscore_pool,
            tc.tile_pool(name="pvecp", bufs=1, space="PSUM") as pvec_pool,
        ):
            # ---------------- weights / constants prep ----------------
            # WhT[p, ko, a] = W[a, ko*128 + p] (d half); WzT likewise (dz half)
            WhT = const_pool.tile([P, DO, D], BF16, tag="WhT")
            WzT = const_pool.tile([P, DO, D], BF16, tag="WzT")
            # zT[p, ko, n] = Z[n, ko*128 + p] for n < BL (padded to 16)
            zT = const_pool.tile([P, DO, 16], BF16, tag="zT")
            b_colT = const_pool.tile([P, DO], F32, tag="b_colT")      # b[ao*128+p]
            v_colT = const_pool.tile([P, DO], BF16, tag="v_colT")     # v[ao*128+p]
            biasvec = const_pool.tile([P, DO, BL], F32, tag="biasvec")

            # W halves -> bf16 DRAM scratch -> XBAR transpose (Wh first so the
            # main matmuls can start while Wz is still in flight)
            Wh_dram = dram_pool.tile([D, D], BF16, tag="wh_dram")
            Wz_dram = dram_pool.tile([D, D], BF16, tag="wz_dram")
            for half, wdram in ((0, Wh_dram), (1, Wz_dram)):
                for ao in range(DO):
                    wstg = stage_pool.tile([P, D], F32, tag="stage")
                    nc.sync.dma_start(
                        out=wstg,
                        in_=W_ext[ao * P : (ao + 1) * P, half * D : (half + 1) * D],
                    )
                    wbf = cbf_pool.tile([P, D], BF16, tag="cbf")
                    nc.vector.tensor_copy(out=wbf, in_=wstg)
                    nc.gpsimd.dma_start(out=wdram[ao * P : (ao + 1) * P, :], in_=wbf)
                nc.scalar.dma_start_transpose(WhT if half == 0 else WzT, wdram[:, :])

            # Z -> bf16 (pad to 16 rows) -> DRAM -> transpose into zT
            zstg = stage_pool.tile([16, D], F32, tag="stage")
            nc.vector.memset(zstg, 0.0)
            nc.sync.dma_start(out=zstg[:BL, :], in_=Z_ext[:, :])
            zbf = cbf_pool.tile([16, D], BF16, tag="cbf")
            nc.vector.tensor_copy(out=zbf, in_=zstg)
            z_dram = dram_pool.tile([16, D], BF16, tag="z_dram")
            nc.gpsimd.dma_start(out=z_dram, in_=zbf)
            nc.scalar.dma_start_transpose(zT, z_dram[:, :])

            # b, v in [a_p, a_o] column layout (DO tiny DMAs each)
            vstg = stage_pool.tile([P, DO], F32, tag="stage")
            for ao in range(DO):
                nc.sync.dma_start(
                    out=b_colT[:, ao : ao + 1],
                    in_=b_ext[ao * P : (ao + 1) * P].rearrange("(p o) -> p o", o=1),
                )
                nc.sync.dma_start(
                    out=vstg[:, ao : ao + 1],
                    in_=v_ext[ao * P : (ao + 1) * P].rearrange("(p o) -> p o", o=1),
                )
            nc.vector.tensor_copy(out=v_colT, in_=vstg)

            # biasvec[a, b] = sum_dz Wz[a, dz] z[b, dz] + bias[a]   (all batches)
            pvec = pvec_pool.tile([P, DO, BL], F32, tag="pvec")
            for ao in range(DO):
                for ko in range(DO):
                    nc.tensor.matmul(
                        pvec[:, ao, :],
                        lhsT=WzT[:, ko, ao * P : (ao + 1) * P],
                        rhs=zT[:, ko, :BL],
                        start=(ko == 0),
                        stop=(ko == DO - 1),
                    )
            nc.vector.tensor_add(
                out=biasvec,
                in0=pvec,
                in1=b_colT[:, :, None].to_broadcast((P, DO, BL)),
            )

            # ---------------- per-batch pipeline ----------------
            for b in range(BL):
                hT = hT_pool.tile([P, DO, T], BF16, tag="hT")

                # load f32 -> cast bf16 -> DRAM quadrant -> XBAR transpose
                for q in range(TQ):
                    hbf_q = dram_pool.tile([QR, D], BF16, tag="hbfq", bufs=2 * TQ)
                    stg = stage_pool.tile([P, QB, D], F32, tag="stage")
                    nc.sync.dma_start(
                        out=stg,
                        in_=h_ext[b, q * QR : (q + 1) * QR, :].rearrange(
                            "(to p) d -> p to d", p=P
                        ),
                    )
                    cb = cbf_pool.tile([P, QB, D], BF16, tag="cbf")
                    nc.vector.tensor_copy(out=cb, in_=stg)
                    nc.gpsimd.dma_start(
                        out=hbf_q[:, :].rearrange("(to p) d -> p to d", p=P), in_=cb
                    )
                    nc.scalar.dma_start_transpose(
                        hT[:, :, q * QR : (q + 1) * QR], hbf_q[:, :]
                    )

                # main matmul [a, t] + fused-bias tanh + score matvec.
                # exp/alpha-broadcast/C are pipelined per chunk with
                # UNNORMALIZED weights; normalization happens once at the end.
                eraw_row = misc_pool.tile([1, T], BF16, tag="eraw_row")
                sums4 = misc_pool.tile([1, NCH], F32, tag="sums4")
                CTpart = misc_pool.tile([P, DO, NCH], F32, tag="CTpart")
                for c in range(NCH):
                    csl = slice(c * CH, (c + 1) * CH)
                    k_c = kc_pool.tile([P, DO, CH], BF16, tag="kc")
                    for ao in range(DO):
                        pre = ppre_pool.tile([P, CH], F32, tag="pre")
                        for ko in range(DO):
                            nc.tensor.matmul(
                                pre,
                                lhsT=WhT[:, ko, ao * P : (ao + 1) * P],
                                rhs=hT[:, ko, csl],
                                start=(ko == 0),
                                stop=(ko == DO - 1),
                            )
                        nc.scalar.activation(
                            out=k_c[:, ao, :], in_=pre, func=Tanh,
                            bias=biasvec[:, ao, b : b + 1],
                        )
                    psc = pscore_pool.tile([1, CH], F32, tag="psc")
                    for ao in range(DO):
                        nc.tensor.matmul(
                            psc,
                            lhsT=v_colT[:, ao : ao + 1],
                            rhs=k_c[:, ao, :],
                            start=(ao == 0),
                            stop=(ao == DO - 1),
                        )
                    # e = exp(score) in bf16 + this chunk's sum (f32)
                    nc.scalar.activation(
                        out=eraw_row[:, csl], in_=psc, func=Exp,
                        accum_out=sums4[:, c : c + 1],
                    )
                    # replicate e across partitions via DRAM stride-0 read
                    e_dram = dram_pool.tile([1, CH], BF16, tag="e_dram", bufs=2 * NCH)
                    nc.sync.dma_start(out=e_dram, in_=eraw_row[:, csl])
                    ed_ap = e_dram[:, :]
                    e_bc = tmpc_pool.tile([P, CH], BF16, tag="ebc", bufs=3)
                    nc.sync.dma_start(
                        out=e_bc,
                        in_=bass.AP(tensor=ed_ap.tensor, offset=ed_ap.offset,
                                    ap=[[0, P], [1, CH]]),
                    )
                    # unnormalized C partials: sum_t hT[d, t] e[t] per chunk
                    for ko in range(DO):
                        tmpc = tmpc_pool.tile([P, CH], BF16, tag="tmpc", bufs=2)
                        nc.vector.tensor_mul(out=tmpc, in0=hT[:, ko, csl], in1=e_bc)
                        nc.vector.tensor_reduce(
                            out=CTpart[:, ko, c : c + 1], in_=tmpc,
                            axis=mybir.AxisListType.X, op=Add,
                        )

                # finalize: total sum, reciprocal, scale partial-C, store
                ssum = misc_pool.tile([1, 1], F32, tag="ssum")
                nc.vector.tensor_reduce(
                    out=ssum, in_=sums4, axis=mybir.AxisListType.X, op=Add
                )
                inv = misc_pool.tile([1, 1], F32, tag="inv")
                nc.vector.reciprocal(out=inv, in_=ssum)
                inv_dram = dram_pool.tile([1, 1], F32, tag="inv_dram")
                nc.sync.dma_start(out=inv_dram, in_=inv)
                iv_ap = inv_dram[:, :]
                inv_bc = misc_pool.tile([P, 1], F32, tag="inv_bc")
                nc.sync.dma_start(
                    out=inv_bc,
                    in_=bass.AP(tensor=iv_ap.tensor, offset=iv_ap.offset,
                                ap=[[0, P], [1, 1]]),
                )
                CT = misc_pool.tile([P, DO], F32, tag="CT")
                nc.vector.tensor_reduce(
                    out=CT.rearrange("p (ko o) -> p ko o", o=1), in_=CTpart,
                    axis=mybir.AxisListType.X, op=Add,
                )
                nc.vector.tensor_scalar_mul(out=CT, in0=CT, scalar1=inv_bc)
                for ko in range(DO):
                    nc.sync.dma_start(
                        out=out_ext[b, ko * P : (ko + 1) * P].rearrange(
                            "(p o) -> p o", o=1
                        ),
                        in_=CT[:, ko : ko + 1],
                    )

    if finalize:
        nc.finalize()
    return nc


def _get_graph():
    global _graph_cache
    if _graph_cache is None:
        _graph_cache = _build()
    return _graph_cache


def kernel(Z_st, h_n_state, W, b, v, _trace=False):
    from concourse.bass_utils import run_bass_kernel_spmd

    nc = _get_graph()
    Z_st = np.ascontiguousarray(np.asarray(Z_st, dtype=np.float32))
    h_n_state = np.ascontiguousarray(np.asarray(h_n_state, dtype=np.float32))
    W = np.ascontiguousarray(np.asarray(W, dtype=np.float32))
    b = np.ascontiguousarray(np.asarray(b, dtype=np.float32))
    v = np.ascontiguousarray(np.asarray(v, dtype=np.float32))

    in_maps = []
    for c in range(NCORES):
        sl = slice(c * BL, (c + 1) * BL)
        in_maps.append(
            {
                "Z_st": Z_st[sl],
                "h_n_state": h_n_state[sl],
                "W": W,
                "b": b,
                "v": v,
            }
        )
    res = run_bass_kernel_spmd(nc, in_maps, core_ids=list(range(NCORES)), trace=_trace)
    out = np.concatenate([res.results[c]["out"] for c in range(NCORES)], axis=0)
    if _trace:
        kernel.last_exec_time_ns = res.exec_time_ns
        kernel.last_results = res
    return out



# revision 2
# speedup vs baseline: 1.4451x; 1.4451x over previous
"""Bahdanau additive attention on 8 TRN2 NeuronCores (data-parallel over batch).

Per batch item b (T=2048 steps, DH=DZ=DA=1024):
    pre[a, t] = sum_d Wh[a, d] h[t, d] + bias_b[a]     (PE, bf16)
                bias_b = Wz @ z_b + b                   (PE, once for all b)
    k         = tanh(pre)                               (ACT, bias fused)
    score[t]  = sum_a v[a] k[a, t]                      (PE matvec)
    e         = exp(score)  (unnormalized; |score| <= sum|v| ~ 23)
    C_raw[d]  = sum_t e[t] h[t, d]                      (DVE mul + reduce)
    C         = C_raw / sum_t e[t]                      (normalized at the end)

Data movement: h enters the tensor engine with d on partitions (hT).
f32 has no XBAR transpose, so h is cast f32->bf16 by a gpsimd DMA
(DRAM->DRAM, cast in flight - no SBUF staging, no DVE pass), then read
back through the DMA XBAR transpose into a chunk-major [128, c, ko, 512]
SBUF layout (contiguous transpose destinations). W/z take the same path.
Transposes are issued from the sync engine so the scalar engine stays
free for tanh/exp.

e is broadcast across partitions with a K=1 ones matmul into PSUM
(no DRAM round trip), copied to bf16 SBUF by ACT, and folded into
C partials by one DVE multiply per chunk (ko-broadcast AP) + 8 reduces.
Normalization happens once at the very end for all 8 batches: one
reciprocal, one f32 ones-matmul broadcast, one elementwise scale, one
f32 PE transpose, and a single contiguous store.

Sharding: batch 64 -> 8 cores x 8 batches. W/b/v replicated. No
collectives; the host concatenates the per-core [8, 1024] outputs.
"""

import os
import sys

for _p in ("/opt/trn_rl_repo", "/root/.axon_site/_ro/trn_rl_repo"):
    if os.path.isdir(_p) and _p not in sys.path:
        sys.path.insert(0, _p)

import numpy as np

P = 128
NCORES = 8
B, T, D = 64, 2048, 1024
BL = B // NCORES          # local batches per core
DO = D // P               # 8 128-blocks in D (d, a, dz alike)
CH = 512                  # matmul t-chunk (= one PSUM bank)
NCH = T // CH

_graph_cache = None


def _build(finalize=True):
    import concourse.bass as bass  # noqa: F401
    import concourse.tile as tile
    import concourse.mybir as mybir
    from concourse import bacc
    from concourse.masks import make_identity

    F32 = mybir.dt.float32
    BF16 = mybir.dt.bfloat16
    Tanh = mybir.ActivationFunctionType.Tanh
    Exp = mybir.ActivationFunctionType.Exp
    Add = mybir.AluOpType.add

    nc = bacc.Bacc(None, target_bir_lowering=False)
    Z_ext = nc.declare_dram_parameter("Z_st", [BL, D], F32, isOutput=False)
    h_ext = nc.declare_dram_parameter("h_n_state", [BL, T, D], F32, isOutput=False)
    W_ext = nc.declare_dram_parameter("W", [D, 2 * D], F32, isOutput=False)
    b_ext = nc.declare_dram_parameter("b", [D], F32, isOutput=False)
    v_ext = nc.declare_dram_parameter("v", [D], F32, isOutput=False)
    out_ext = nc.declare_dram_parameter("out", [BL, D], F32, isOutput=True)

    with tile.TileContext(nc) as tc:
        with (
            tc.tile_pool(name="const", bufs=1) as const_pool,
            tc.tile_pool(name="hT", bufs=2) as hT_pool,
            tc.tile_pool(name="kc", bufs=2) as kc_pool,
            tc.tile_pool(name="ebc_sb", bufs=2) as ebc_sb_pool,
            tc.tile_pool(name="tmp", bufs=2) as tmp_pool,
            tc.tile_pool(name="misc", bufs=2) as misc_pool,
            tc.tile_pool(name="dram", bufs=1, space="DRAM") as dram_pool,
            tc.tile_pool(name="hbf", bufs=3, space="DRAM") as hbf_pool,
            tc.tile_pool(name="ppre", bufs=3, space="PSUM") as ppre_pool,
            tc.tile_pool(name="pscore", bufs=2, space="PSUM") as pscore_pool,
            tc.tile_pool(name="pebc", bufs=2, space="PSUM") as pebc_pool,
            tc.tile_pool(name="pmisc", bufs=1, space="PSUM") as pmisc_pool,
        ):
            # ---------------- weights / constants prep ----------------
            # cast f32 -> bf16 in-flight, DRAM -> DRAM (gpsimd SWDGE)
            Wz_bf = dram_pool.tile([D, D], BF16, tag="wz_bf")
            Wh_bf = dram_pool.tile([D, D], BF16, tag="wh_bf")
            z_bf = dram_pool.tile([16, D], BF16, tag="z_bf")
            nc.gpsimd.dma_start(out=Wz_bf, in_=W_ext[:, D : 2 * D])
            nc.gpsimd.dma_start(out=Wh_bf, in_=W_ext[:, 0:D])
            nc.gpsimd.dma_start(out=z_bf[0:BL, :], in_=Z_ext[:, :])

            # XBAR transposes (sync engine; ACT stays free for tanh/exp)
            # WhT[p, ko, a] = W[a, ko*128 + p] (d half); WzT the dz half.
            WzT = const_pool.tile([P, DO, D], BF16, tag="WzT")
            WhT = const_pool.tile([P, DO, D], BF16, tag="WhT")
            zT = const_pool.tile([P, DO, 16], BF16, tag="zT")
            nc.sync.dma_start_transpose(WzT, Wz_bf[:, :])
            nc.sync.dma_start_transpose(WhT, Wh_bf[:, :])
            nc.sync.dma_start_transpose(zT, z_bf[:, :])

            # v in [a_p, a_o] column layout; b as a bf16 row (folded into
            # the biasvec matmul as a K=1 rank-1 update)
            vstg = const_pool.tile([P, DO], F32, tag="vstg")
            for ao in range(DO):
                nc.sync.dma_start(
                    out=vstg[:, ao : ao + 1],
                    in_=v_ext[ao * P : (ao + 1) * P].rearrange("(p o) -> p o", o=1),
                )
            v_colT = const_pool.tile([P, DO], BF16, tag="v_colT")
            nc.vector.tensor_copy(out=v_colT, in_=vstg)

            b_row = const_pool.tile([1, D], F32, tag="b_row")
            nc.sync.dma_start(out=b_row, in_=b_ext.rearrange("(o d) -> o d", o=1))
            b_bf = const_pool.tile([1, D], BF16, tag="b_bf")
            nc.vector.tensor_copy(out=b_bf, in_=b_row)

            ones_bf = const_pool.tile([1, P], BF16, tag="ones_bf")
            nc.vector.memset(ones_bf, 1.0)
            ones_f32 = const_pool.tile([1, P], F32, tag="ones_f32")
            nc.vector.memset(ones_f32, 1.0)
            ident = const_pool.tile([P, P], F32, tag="ident")
            make_identity(nc, ident)

            # biasvec[a, b] = sum_dz Wz[a, dz] z[b, dz] + b[a]
            pvec = pmisc_pool.tile([P, DO, BL], F32, tag="pmisc")
            for ao in range(DO):
                aosl = slice(ao * P, (ao + 1) * P)
                for ko in range(DO):
                    nc.tensor.matmul(
                        pvec[:, ao, :],
                        lhsT=WzT[:, ko, aosl],
                        rhs=zT[:, ko, :BL],
                        start=(ko == 0),
                        stop=False,
                    )
                nc.tensor.matmul(
                    pvec[:, ao, :],
                    lhsT=b_bf[0:1, aosl],
                    rhs=ones_bf[0:1, :BL],
                    start=False,
                    stop=True,
                )
            biasvec = const_pool.tile([P, DO, BL], F32, tag="biasvec")
            nc.vector.tensor_copy(out=biasvec, in_=pvec)

            # accumulators shared across batches
            sums_all = const_pool.tile([1, BL, NCH], F32, tag="sums_all")
            CT_all = const_pool.tile([P, BL, DO], F32, tag="CT_all")

            # ---------------- per-batch pipeline ----------------
            for b in range(BL):
                # cast + transpose, quadrant at a time (chunk-major dest)
                hbf = hbf_pool.tile([T, D], BF16, tag="hbf")
                hT = hT_pool.tile([P, NCH, DO, CH], BF16, tag="hT")
                for c in range(NCH):
                    rsl = slice(c * CH, (c + 1) * CH)
                    nc.gpsimd.dma_start(out=hbf[rsl, :], in_=h_ext[b, rsl, :])
                    nc.sync.dma_start_transpose(hT[:, c], hbf[rsl, :])

                eraw = misc_pool.tile([1, T], BF16, tag="eraw")
                CTpart = misc_pool.tile([P, DO, NCH], F32, tag="CTpart")
                for c in range(NCH):
                    csl = slice(c * CH, (c + 1) * CH)
                    # pre = Wh @ hT (+bias) -> k = tanh
                    kc = kc_pool.tile([P, DO, CH], BF16, tag="kc")
                    for ao in range(DO):
                        pre = ppre_pool.tile([P, CH], F32, tag="pre")
                        for ko in range(DO):
                            nc.tensor.matmul(
                                pre,
                                lhsT=WhT[:, ko, ao * P : (ao + 1) * P],
                                rhs=hT[:, c, ko],
                                start=(ko == 0),
                                stop=(ko == DO - 1),
                            )
                        nc.scalar.activation(
                            out=kc[:, ao, :], in_=pre, func=Tanh,
                            bias=biasvec[:, ao, b : b + 1],
                        )
                    # score = v . k  (PE matvec)
                    psc = pscore_pool.tile([1, CH], F32, tag="psc")
                    for ao in range(DO):
                        nc.tensor.matmul(
                            psc,
                            lhsT=v_colT[:, ao : ao + 1],
                            rhs=kc[:, ao],
                            start=(ao == 0),
                            stop=(ao == DO - 1),
                        )
                    # e = exp(score) (bf16) + chunk sum (f32)
                    nc.scalar.activation(
                        out=eraw[:, csl], in_=psc, func=Exp,
                        accum_out=sums_all[0:1, b, c : c + 1],
                    )
                    # broadcast e across partitions via K=1 ones matmul
                    ebc = pebc_pool.tile([P, CH], F32, tag="ebc")
                    nc.tensor.matmul(
                        ebc, lhsT=ones_bf[0:1, :], rhs=eraw[0:1, csl],
                        start=True, stop=True,
                    )
                    ebc_sb = ebc_sb_pool.tile([P, CH], BF16, tag="ebc_sb")
                    nc.scalar.copy(out=ebc_sb, in_=ebc)
                    # C partials: one mul over all ko, then per-ko reduce
                    tmp = tmp_pool.tile([P, DO, CH], BF16, tag="tmp")
                    nc.vector.tensor_mul(
                        out=tmp, in0=hT[:, c],
                        in1=ebc_sb[:, None, :].to_broadcast((P, DO, CH)),
                    )
                    for ko in range(DO):
                        nc.vector.tensor_reduce(
                            out=CTpart[:, ko, c : c + 1], in_=tmp[:, ko],
                            axis=mybir.AxisListType.X, op=Add,
                        )
                # fold the 4 chunk partials
                nc.vector.tensor_reduce(
                    out=CT_all[:, b, :], in_=CTpart,
                    axis=mybir.AxisListType.X, op=Add,
                )

            # ---------------- finalize all batches ----------------
            ssum = const_pool.tile([1, BL], F32, tag="ssum")
            nc.vector.tensor_reduce(
                out=ssum, in_=sums_all, axis=mybir.AxisListType.X, op=Add
            )
            inv = const_pool.tile([1, BL], F32, tag="inv")
            nc.vector.reciprocal(out=inv, in_=ssum)
            pinv = pmisc_pool.tile([P, BL], F32, tag="pmisc")
            nc.tensor.matmul(
                pinv, lhsT=ones_f32[0:1, :], rhs=inv[0:1, :], start=True, stop=True
            )
            nc.vector.tensor_mul(
                out=CT_all, in0=CT_all,
                in1=pinv[:, :, None].to_broadcast((P, BL, DO)),
            )
            # transpose [d_p, (b ko)] -> [(b ko), d_p] and store contiguously
            ptr = pmisc_pool.tile([BL * DO, P], F32, tag="pmisc")
            nc.tensor.transpose(
                ptr, CT_all.rearrange("p b k -> p (b k)"), ident
            )
            tr_sb = const_pool.tile([BL * DO, P], F32, tag="tr_sb")
            nc.vector.tensor_copy(out=tr_sb, in_=ptr)
            nc.sync.dma_start(
                out=out_ext.rearrange("b (k p) -> (b k) p", p=P), in_=tr_sb
            )

    if finalize:
        nc.finalize()
    return nc


def _get_graph():
    global _graph_cache
    if _graph_cache is None:
        _graph_cache = _build()
    return _graph_cache


def kernel(Z_st, h_n_state, W, b, v, _trace=False):
    from concourse.bass_utils import run_bass_kernel_spmd

    nc = _get_graph()
    Z_st = np.ascontiguousarray(np.asarray(Z_st, dtype=np.float32))
    h_n_state = np.ascontiguousarray(np.asarray(h_n_state, dtype=np.float32))
    W = np.ascontiguousarray(np.asarray(W, dtype=np.float32))
    b = np.ascontiguousarray(np.asarray(b, dtype=np.float32))
    v = np.ascontiguousarray(np.asarray(v, dtype=np.float32))

    in_maps = []
    for c in range(NCORES):
        sl = slice(c * BL, (c + 1) * BL)
        in_maps.append(
            {
                "Z_st": Z_st[sl],
                "h_n_state": h_n_state[sl],
                "W": W,
                "b": b,
                "v": v,
            }
        )
    res = run_bass_kernel_spmd(nc, in_maps, core_ids=list(range(NCORES)), trace=_trace)
    out = np.concatenate([res.results[c]["out"] for c in range(NCORES)], axis=0)
    if _trace:
        kernel.last_exec_time_ns = res.exec_time_ns
        kernel.last_results = res
    return out
